# revision 14
# baseline (speedup 1.0000x reference)
"""Sliding-window attention (L=4096, H=2048, 16 heads, window 1024) on 8 TRN2 cores.

Collective-free sequence sharding: core c owns query rows [512c, 512c+512).
Each core receives hs rows [512c-1024, 512c+512) pre-transposed ([H, WROWS],
zero-padded for cores 0/1) and computes K/V projections for that window
itself, so no cross-core communication is needed. Heads are processed in a
16-iteration streaming loop (weight columns streamed per head) to bound SBUF.

Engine split: PE does matmuls only; ACT does exps + output copies; DVE does
PSUM->SBUF copies, masks and normalization; Pool does the RoPE arithmetic.
Softmax exps are batched per window k-tile (one variable-width exp per
k-tile covering all valid q-tiles). All matmuls run in bf16 (fp32 PSUM).
"""

import sys

import numpy as np

if "/opt/trn_rl_repo" not in sys.path:
    sys.path.insert(0, "/opt/trn_rl_repo")

L = 4096
H = 2048
NH = 16
D = 128
WIN = 1024
NCORES = 8
QROWS = L // NCORES          # 512 query rows per core
WROWS = QROWS + WIN          # 1536 window rows per core
NQT = QROWS // 128           # 4 q tiles per core
NWT = WROWS // 128           # 12 window k tiles per core
NKT = 9                      # k tiles attended per q tile
ROPE_THETA = 10000.0
SCALE = float(D) ** -0.5
NEG = -1e30

# per window k-tile kt: valid q tiles are lq in [kt-8, kt] ∩ [0, NQT)
_LQ0 = [max(0, kt - (NKT - 1)) for kt in range(NWT)]
_NV = [min(NQT - 1, kt) - _LQ0[kt] + 1 for kt in range(NWT)]
_OFF = [sum(_NV[:kt]) for kt in range(NWT)]
NPB = sum(_NV)               # 36 p-tile blocks per head

_CACHE = {}


def _trace(tc, aps):
    from contextlib import ExitStack

    from concourse import mybir

    nc = tc.nc
    f32 = mybir.dt.float32
    bf16 = mybir.dt.bfloat16
    AF = mybir.ActivationFunctionType
    hswT, wq, wk, wv, wo, cosw, sinw, tsel2, maskl, maskd, idb, out = aps

    ctx = ExitStack()
    const = ctx.enter_context(tc.tile_pool(name="const", bufs=1))
    hstp = ctx.enter_context(tc.tile_pool(name="hst", bufs=1))
    otp = ctx.enter_context(tc.tile_pool(name="otp", bufs=1))
    wstr = ctx.enter_context(tc.tile_pool(name="wstr", bufs=2))
    kvp = ctx.enter_context(tc.tile_pool(name="kvp", bufs=2))
    rope = ctx.enter_context(tc.tile_pool(name="rope", bufs=4))
    attn = ctx.enter_context(tc.tile_pool(name="attn", bufs=4))
    pbp = ctx.enter_context(tc.tile_pool(name="pbp", bufs=2))
    phc = ctx.enter_context(tc.tile_pool(name="phc", bufs=2))
    actx = ExitStack()  # attention-phase PSUM pools, closed before o_proj
    ps_b = actx.enter_context(tc.tile_pool(name="ps_b", bufs=2, space="PSUM"))
    ps_st = actx.enter_context(tc.tile_pool(name="ps_st", bufs=2, space="PSUM"))
    ps_v = actx.enter_context(tc.tile_pool(name="ps_v", bufs=2, space="PSUM"))
    ps_o = actx.enter_context(tc.tile_pool(name="ps_o", bufs=1, space="PSUM"))
    ps_t = actx.enter_context(tc.tile_pool(name="ps_t", bufs=1, space="PSUM"))

    # ---- constants ----
    maskl_sb = const.tile([128, 128], f32, name="maskl_sb")
    nc.sync.dma_start(out=maskl_sb, in_=maskl)
    maskd_sb = const.tile([128, 128], f32, name="maskd_sb")
    nc.sync.dma_start(out=maskd_sb, in_=maskd)
    idb_sb = const.tile([128, 128], bf16, name="idb_sb")
    nc.sync.dma_start(out=idb_sb, in_=idb)
    cos_sb = const.tile([128, WROWS], bf16, name="cos_sb")
    nc.sync.dma_start(out=cos_sb, in_=cosw)
    sin_sb = const.tile([128, WROWS], bf16, name="sin_sb")
    nc.sync.dma_start(out=sin_sb, in_=sinw)
    tsel_sb = const.tile([128, NWT], f32, name="tsel_sb")
    nc.sync.dma_start(out=tsel_sb, in_=tsel2)

    # attention outputs, transposed: [feat-part, kt(=head), q-tile, row]
    ot_sb = otp.tile([128, 16, NQT, 128], bf16, name="ot_sb")

    def load_w(h):
        whs = []
        for w_dram, wtag in ((wq, "wq_h"), (wk, "wk_h"), (wv, "wv_h")):
            w_b = wstr.tile([128, 16, 128], bf16, tag=wtag)
            nc.sync.dma_start(out=w_b, in_=w_dram[:, h, :, :])
            whs.append(w_b)
        return whs

    w_next = load_w(0)

    # ---- load pre-transposed hs window (bf16, [H, WROWS] in DRAM) ----
    # column-chunk major so the first K projection chunk can start after
    # ~1/3 of the bytes have landed
    hsT = hstp.tile([128, 16, WROWS], bf16, name="hsT")
    for rb in range(3):
        for kt in range(16):
            nc.sync.dma_start(
                out=hsT[:, kt, rb * 512:(rb + 1) * 512],
                in_=hswT[kt * 128:(kt + 1) * 128, rb * 512:(rb + 1) * 512],
            )

    def rope_pair(dst, src_ps, c0):
        """RoPE: dst[d, r] = src[d, r]*cos[d, c0+r] + src[(d+64)%128, r]*sin[d, c0+r].
        dst/src are [128, 512]; sin carries the sign for the lower half.
        The half-rotated term reads PSUM directly (mixed-base-partition SBUF
        reads are rejected by the walrus verifier; PSUM+SBUF is allowed)."""
        cols = slice(c0, c0 + 512)
        t1 = rope.tile([128, 512], bf16, tag="t1")
        nc.vector.tensor_mul(t1, src_ps, cos_sb[:, cols])
        t2 = rope.tile([128, 512], bf16, tag="t2")
        nc.vector.tensor_mul(t2[0:64, :], src_ps[64:128, :], sin_sb[0:64, cols])
        nc.vector.tensor_mul(t2[64:128, :], src_ps[0:64, :], sin_sb[64:128, cols])
        nc.gpsimd.tensor_add(dst, t1, t2)

    # ---- per-head stream, PV pipelined one head behind ----
    prev = None  # (h-1)'s (p_sb, v_h) for the delayed PV stage

    def emit_pv(h, lq, p_sb, v_h):
        o_ps = ps_o.tile([128, 132], f32, tag="o", name=f"o{h}_{lq}")
        for t in range(NKT):
            kt = lq + t
            slot = lq - _LQ0[kt]
            nc.tensor.matmul(
                o_ps[:, 0:129],
                lhsT=p_sb[:, _OFF[kt] + slot, :],
                rhs=v_h[:, kt, 0:129],
                start=(t == 0),
                stop=(t == NKT - 1),
            )
        rinv = attn.tile([128, 1], f32, tag="rinv")
        nc.vector.reciprocal(rinv, o_ps[:, 128:129])
        ao = attn.tile([128, 128], bf16, tag="ao")
        nc.vector.tensor_scalar_mul(ao, o_ps[:, 0:128], rinv)
        tp = ps_t.tile([128, 128], bf16, tag="t", name=f"aot{h}_{lq}")
        nc.tensor.transpose(tp, ao, idb_sb)
        nc.scalar.copy(ot_sb[:, h, lq, :], tp)

    for h in range(NH):
        wq_h, wk_h, wv_h = w_next
        if h + 1 < NH:
            w_next = load_w(h + 1)

        # kT for the full window (3 chunks of 512 rows), with RoPE
        kr_h = kvp.tile([128, NWT, 128], bf16, tag="kr_h")
        for rb in range(3):
            ps = ps_b.tile([128, 512], f32, tag="b", name=f"kp{h}_{rb}")
            for kt in range(16):
                nc.tensor.matmul(
                    ps,
                    lhsT=wk_h[:, kt, :],
                    rhs=hsT[:, kt, rb * 512:(rb + 1) * 512],
                    start=(kt == 0),
                    stop=(kt == 15),
                )
            dst = kr_h[:, rb * 4:(rb + 1) * 4, :].rearrange("p a b -> p (a b)")
            rope_pair(dst, ps, rb * 512)

        # qT for the core's own rows (= window rows [1024, 1536)), with RoPE
        qr_h = kvp.tile([128, NQT, 128], bf16, tag="qr_h")
        ps = ps_b.tile([128, 512], f32, tag="b", name=f"qp{h}")
        for kt in range(16):
            nc.tensor.matmul(
                ps,
                lhsT=wq_h[:, kt, :],
                rhs=hsT[:, kt, 1024:1536],
                start=(kt == 0),
                stop=(kt == 15),
            )
        rope_pair(qr_h.rearrange("p a b -> p (a b)"), ps, 1024)

        # V in natural [row, d] orientation (+ ones column for the softmax
        # denominator); (h-1)'s PV stages are interleaved as PE filler
        v_h = kvp.tile([128, NWT, 130], bf16, tag="v_h")
        nc.vector.memset(v_h[:, :, 128:130], 0.0)
        nc.vector.memset(v_h[:, :, 128:129], 1.0)
        for jt in range(NWT):
            ps = ps_v.tile([128, 128], f32, tag="v", name=f"vp{h}_{jt}")
            for kt in range(16):
                nc.tensor.matmul(
                    ps,
                    lhsT=hsT[:, kt, jt * 128:(jt + 1) * 128],
                    rhs=wv_h[:, kt, :],
                    start=(kt == 0),
                    stop=(kt == 15),
                )
            nc.vector.tensor_copy(v_h[:, jt, 0:128], ps)
            if prev is not None and jt % 3 == 2:
                emit_pv(h - 1, jt // 3, *prev)

        # S^T + exp, batched per window k-tile across its valid q-tiles
        p_sb = pbp.tile([128, NPB, 128], bf16, tag="p_sb")
        for kt in range(NWT):
            lq0, nv = _LQ0[kt], _NV[kt]
            st = ps_st.tile([128, 512], f32, tag="st", name=f"st{h}_{kt}")
            for slot in range(nv):
                lq = lq0 + slot
                nc.tensor.matmul(
                    st[:, slot * 128:(slot + 1) * 128],
                    lhsT=kr_h[:, kt, :],
                    rhs=qr_h[:, lq, :],
                    start=True,
                    stop=True,
                )
            if kt < NQT:  # left window edge tile for q-tile lq = kt
                c = (kt - lq0) * 128
                nc.vector.tensor_add(
                    st[:, c:c + 128], st[:, c:c + 128], maskl_sb)
            if kt >= NKT - 1:  # diagonal tile for q-tile lq = kt-8
                c = (kt - (NKT - 1) - lq0) * 128
                nc.vector.tensor_add(
                    st[:, c:c + 128], st[:, c:c + 128], maskd_sb)
            nc.scalar.activation(
                p_sb[:, _OFF[kt]:_OFF[kt] + nv, :]
                .rearrange("p a b -> p (a b)"),
                st[:, 0:nv * 128], AF.Exp,
                bias=tsel_sb[:, kt:kt + 1], scale=SCALE,
            )
        prev = (p_sb, v_h)

    for lq in range(NQT):  # last head's PV
        emit_pv(NH - 1, lq, *prev)

    # ---- o_proj: out[rows, :] = ot.T @ wo, streaming wo once (bf16).
    # All 4 q-tiles accumulate against each streamed wo tile.
    actx.close()
    ps_op = ctx.enter_context(tc.tile_pool(name="ps_op", bufs=2, space="PSUM"))
    for nb in range(4):
        pss = [
            ps_op.tile([128, 512], f32, tag=f"op{i}", name=f"op{nb}_{i}")
            for i in range(4)
        ]
        for kt in range(16):
            wos = phc.tile([128, 512], bf16, tag="wos", bufs=10)
            nc.sync.dma_start(
                out=wos,
                in_=wo[kt * 128:(kt + 1) * 128, nb * 512:(nb + 1) * 512],
            )
            for i in range(4):
                nc.tensor.matmul(
                    pss[i], lhsT=ot_sb[:, kt, i, :], rhs=wos,
                    start=(kt == 0), stop=(kt == 15),
                )
        for i in range(4):
            ob = phc.tile([128, 512], f32, tag="ob", bufs=4)
            nc.scalar.copy(ob, pss[i])
            nc.sync.dma_start(
                out=out[i, :, nb * 512:(nb + 1) * 512], in_=ob
            )

    ctx.close()


def _build(timing=False):
    """Build the module. With timing=True, all real tensors become Internal
    DRAM (garbage contents, valid timing) and tiny dummy ExternalInput/Output
    tensors are added, so benchmarking excludes host<->device transfer."""
    import concourse.bacc as bacc
    import concourse.tile as tile
    from concourse import mybir

    f32 = mybir.dt.float32
    bf16 = mybir.dt.bfloat16

    nc = bacc.Bacc("TRN2", target_bir_lowering=False, debug=False,
                   num_devices=NCORES)
    kind = {} if timing else {"kind": "ExternalInput"}
    okind = {} if timing else {"kind": "ExternalOutput"}
    aps = [
        nc.dram_tensor("hswT", [H, WROWS], bf16, **kind).ap(),
        nc.dram_tensor("wq", [128, NH, 16, 128], bf16, **kind).ap(),
        nc.dram_tensor("wk", [128, NH, 16, 128], bf16, **kind).ap(),
        nc.dram_tensor("wv", [128, NH, 16, 128], bf16, **kind).ap(),
        nc.dram_tensor("wo", [H, H], bf16, **kind).ap(),
        nc.dram_tensor("cosw", [D, WROWS], bf16, **kind).ap(),
        nc.dram_tensor("sinw", [D, WROWS], bf16, **kind).ap(),
        nc.dram_tensor("tsel2", [D, NWT], f32, **kind).ap(),
        nc.dram_tensor("maskl", [128, 128], f32, **kind).ap(),
        nc.dram_tensor("maskd", [128, 128], f32, **kind).ap(),
        nc.dram_tensor("idb", [128, 128], bf16, **kind).ap(),
        nc.dram_tensor("out", [NQT, 128, H], f32, **okind).ap(),
    ]
    dummies = None
    if timing:
        dummies = (
            nc.dram_tensor("dummy_in", [1, 8], f32, kind="ExternalInput").ap(),
            nc.dram_tensor("dummy_out", [1, 8], f32, kind="ExternalOutput").ap(),
        )
    with tile.TileContext(nc) as tc:
        _trace(tc, aps)
        if dummies is not None:
            with tc.tile_pool(name="dummy", bufs=1) as dp:
                dt_ = dp.tile([1, 8], f32, name="dummy_sb")
                nc.sync.dma_start(out=dt_, in_=dummies[0])
                nc.sync.dma_start(out=dummies[1], in_=dt_)
    nc.compile()
    return nc


def bench_device(iters=50):
    """Marginal per-iteration time of the compute with dummy-sized I/O.

    Includes the fixed axon dispatch floor (~7 ms) but not the big-tensor
    relay transfers; deltas between kernel variants reflect device time.
    """
    if "timing_runner" not in _CACHE:
        tnc = _build(timing=True)
        _CACHE["timing_runner"] = _Runner(tnc)
    r = _CACHE["timing_runner"]
    maps = [{"dummy_in": np.zeros((1, 8), np.float32)} for _ in range(NCORES)]
    return r.bench(maps, iters=iters)


def _host_constants():
    import ml_dtypes

    inv = 1.0 / (ROPE_THETA ** (np.arange(0, D, 2, dtype=np.float64) / D))
    ii = np.arange(128)
    # masks for S^T [j, i] tiles; valid -> 0, invalid -> NEG
    maskl = np.where(ii[:, None] > ii[None, :], 0.0, NEG).astype(np.float32)
    maskd = np.where(ii[:, None] <= ii[None, :], 0.0, NEG).astype(np.float32)
    idb = np.eye(128).astype(ml_dtypes.bfloat16)

    cos_list, sin_list, tsel_list = [], [], []
    for c in range(NCORES):
        # window rows are global positions [512c - 1024, 512c + 512)
        pos = np.arange(c * QROWS - WIN, c * QROWS + QROWS, dtype=np.float64)
        pos = np.maximum(pos, 0.0)         # pad rows: value irrelevant (masked)
        ang = inv[:, None] * pos[None, :]  # [64, WROWS]
        cos_list.append(np.concatenate([np.cos(ang), np.cos(ang)], 0)
                        .astype(ml_dtypes.bfloat16))
        sin_list.append(np.concatenate([-np.sin(ang), np.sin(ang)], 0)
                        .astype(ml_dtypes.bfloat16))
        # tsel2[kt] = 0 if local window k-tile kt is a real tile else NEG
        ts = np.zeros((NWT,), np.float32)
        for kt in range(NWT):
            if (c * QROWS - WIN) // 128 + kt < 0:
                ts[kt] = NEG
        tsel_list.append(np.broadcast_to(ts, (128, NWT)).copy())
    return cos_list, sin_list, tsel_list, maskl, maskd, idb


def _get_state():
    if "nc" not in _CACHE:
        _CACHE["nc"] = _build()
        _CACHE["consts"] = _host_constants()
    return _CACHE["nc"], _CACHE["consts"]


def _in_maps(hidden_states, wq, wk, wv, wo, consts):
    import ml_dtypes

    bf16 = ml_dtypes.bfloat16
    hs = np.asarray(hidden_states, np.float32).reshape(L, H).astype(bf16)

    def w_rearrange(w):
        # [kt*128+p, h*128+f] -> [p, h, kt, f] so per-head loads are
        # contiguous 4KB-per-partition DMA lines
        w = np.asarray(w, np.float32).astype(bf16)
        return np.ascontiguousarray(
            w.reshape(16, 128, NH, 128).transpose(1, 2, 0, 3))

    wq = w_rearrange(wq)
    wk = w_rearrange(wk)
    wv = w_rearrange(wv)
    wo = np.asarray(wo, np.float32).astype(bf16)
    cos_list, sin_list, tsel_list, maskl, maskd, idb = consts
    maps = []
    for c in range(NCORES):
        lo = c * QROWS - WIN
        hsw = np.zeros((WROWS, H), bf16)
        src_lo = max(lo, 0)
        hsw[src_lo - lo:] = hs[src_lo:c * QROWS + QROWS]
        maps.append({
            "hswT": np.ascontiguousarray(hsw.T),
            "wq": wq,
            "wk": wk,
            "wv": wv,
            "wo": wo,
            "cosw": cos_list[c],
            "sinw": sin_list[c],
            "tsel2": tsel_list[c],
            "maskl": maskl,
            "maskd": maskd,
            "idb": idb,
        })
    return maps


def _gather(results):
    full = np.empty((L, H), np.float32)
    for c in range(NCORES):
        full[c * QROWS:(c + 1) * QROWS] = results[c]["out"].reshape(QROWS, H)
    return full.reshape(1, L, H)


class _Runner:
    """Persistent jitted shard_map executable over the 8 axon cores.

    Mirrors bass2jax.run_bass_via_pjrt's multi-core path, but builds the
    jitted callable once (so repeat kernel() calls skip retracing) and
    skips output-buffer donation (this kernel writes every output element,
    so the pre-zeroed-output contract is not needed).
    """

    def __init__(self, nc):
        import jax
        from jax.sharding import Mesh, PartitionSpec
        from jax.experimental.shard_map import shard_map
        from concourse import mybir
        from concourse import bass2jax

        bass2jax.install_neuronx_cc_hook()

        partition_name = (
            nc.partition_id_tensor.name if nc.partition_id_tensor else None
        )
        in_names, out_names, out_avals, zero_outs = [], [], [], []
        for alloc in nc.m.functions[0].allocations:
            if not isinstance(alloc, mybir.MemoryLocationSet):
                continue
            name = alloc.memorylocations[0].name
            if alloc.kind == "ExternalInput":
                if name != partition_name:
                    in_names.append(name)
            elif alloc.kind == "ExternalOutput":
                out_names.append(name)
                shape = tuple(alloc.tensor_shape)
                dtype = mybir.dt.np(alloc.dtype)
                out_avals.append(jax.core.ShapedArray(shape, dtype))
                zero_outs.append(np.zeros(shape, dtype))
        self.n_params = len(in_names)
        self.in_names = list(in_names)
        self.out_names = out_names
        all_names = in_names + out_names
        if partition_name is not None:
            all_names = all_names + [partition_name]

        def _body(*args):
            operands = list(args)
            if partition_name is not None:
                operands.append(bass2jax.partition_id_tensor())
            outs = bass2jax._bass_exec_p.bind(
                *operands,
                out_avals=tuple(out_avals),
                in_names=tuple(all_names),
                out_names=tuple(out_names),
                lowering_input_output_aliases=(),
                sim_require_finite=True,
                sim_require_nnan=True,
                nc=nc,
            )
            return tuple(outs)

        devices = jax.devices()[:NCORES]
        assert len(devices) == NCORES
        self.mesh = Mesh(np.asarray(devices), ("core",))
        in_specs = (PartitionSpec("core"),) * (self.n_params + len(out_names))
        out_specs = (PartitionSpec("core"),) * len(out_names)
        self.sharded = jax.jit(
            shard_map(_body, mesh=self.mesh, in_specs=in_specs,
                      out_specs=out_specs, check_rep=False),
            keep_unused=True,
        )
        self.out_avals = out_avals
        self.concat_zeros = [
            np.zeros((NCORES * z.shape[0], *z.shape[1:]), z.dtype)
            for z in zero_outs
        ]
        self._dev_args = None

    def pack(self, maps):
        return [
            np.concatenate([np.asarray(maps[c][n]) for c in range(NCORES)], axis=0)
            for n in self.in_names
        ]

    def run(self, maps):
        import jax

        concat_in = self.pack(maps)
        out_arrs = self.sharded(*concat_in, *self.concat_zeros)
        return [
            {
                n: np.asarray(out_arrs[i]).reshape(
                    NCORES, *self.out_avals[i].shape)[c]
                for i, n in enumerate(self.out_names)
            }
            for c in range(NCORES)
        ]

    def bench(self, maps, iters=10):
        """Time repeated executions with inputs resident on device."""
        import time

        import jax

        args = [jax.device_put(a) for a in self.pack(maps)]
        args += [jax.device_put(z) for z in self.concat_zeros]
        out = self.sharded(*args)  # warm
        jax.block_until_ready(out)
        t0 = time.perf_counter()
        for _ in range(iters):
            out = self.sharded(*args)
        jax.block_until_ready(out)
        return (time.perf_counter() - t0) / iters


def _get_runner():
    nc, consts = _get_state()
    if "runner" not in _CACHE:
        _CACHE["runner"] = _Runner(nc)
    return _CACHE["runner"], consts


def kernel(hidden_states, wq, wk, wv, wo):
    runner, consts = _get_runner()
    maps = _in_maps(hidden_states, wq, wk, wv, wo, consts)
    return _gather(runner.run(maps))


def bench(hidden_states, wq, wk, wv, wo, iters=10):
    runner, consts = _get_runner()
    maps = _in_maps(hidden_states, wq, wk, wv, wo, consts)
    return runner.bench(maps, iters=iters)
